# revision 75
# baseline (speedup 1.0000x reference)
"""AverageAttention Trainium2 kernel.

Computes, per batch b (data-parallel across 8 NeuronCores):
    avg      = cumsum(x, axis=seq) / (pos+1)
    inter    = relu(LN(avg) @ w1 + b1)
    avg_out  = inter @ w2 + b2 + avg
    gates    = [x, avg_out] @ wg + bg
    gated    = sigmoid(gates[:, :D]) * x + sigmoid(gates[:, D:]) * avg_out
returns (gated, avg_out), each [B, S, D].

Implementation notes:
  - cumsum via triangular matmul per 128-seq block (fp32r streaming) + a K=1
    rank-1 matmul adding the running carry into PSUM, scaled by 1/(pos+1) at
    eviction (per-partition scale on ScalarE). The serial carry chain is a
    single fused DVE tensor_tensor per half (prev + tot-PSUM).
  - LayerNorm gain/bias are folded into w1/b1 on the host; on-chip LN is just
    (x-mu)*rstd via bn_stats/bn_aggr + one tensor_scalar (rstd via DVE-only
    fast-inverse-sqrt so no extra ACT table set is needed).
  - FFN and gating matmuls run in fp8e4 (e4m3) with DoubleRow perf mode
    (2 K-chunks per instruction). Weights are host-scaled by WS=16 to avoid
    fp8 subnormals; descaled at PSUM eviction via ACT scale.
  - Activations are transposed on the PE straight from fp32r (no cast) with a
    bf16 identity; PSUM transposes evict to fp8 SBUF on ScalarE.
  - x stays resident in SBUF for the whole kernel (no gating-phase re-read);
    avg_out is re-read from DRAM for the final elementwise gating.
"""

import os
import sys

if "/opt/trn_rl_repo" not in sys.path:
    sys.path.insert(0, "/opt/trn_rl_repo")

# The NEFF executes via the axon-tunneled PJRT backend; a JAX_PLATFORMS=cpu
# pin (used for running references) would hide the NeuronCores.
if os.environ.get("JAX_PLATFORMS") == "cpu":
    os.environ.pop("JAX_PLATFORMS")

from contextlib import ExitStack

import ml_dtypes
import numpy as np

import concourse.bass as bass
import concourse.mybir as mybir
import concourse.tile as tile
from concourse import bacc
from concourse.bass_utils import run_bass_kernel_spmd

B, S, D = 8, 2048, 1024
P = 128
NBLK = S // P            # 16 seq blocks per core
CB = 2                   # seq blocks per pipeline chunk
NCHUNK = NBLK // CB
CS = CB * P              # chunk seq length (256)
D2 = 2 * D
KC = D // P              # 8 feature chunks of 128
EPS = 1e-6

FP32 = mybir.dt.float32
BF16 = mybir.dt.bfloat16
F32R = mybir.dt.float32r
FP8 = mybir.dt.float8e4
WS = 16.0                     # host-side weight scale (keeps fp8 normal-range)

AF = mybir.ActivationFunctionType
ALU = mybir.AluOpType
DR = mybir.MatmulPerfMode.DoubleRow


def build_program(has_b2: bool, has_bg: bool) -> bacc.Bacc:
    nc = bacc.Bacc("TRN2", target_bir_lowering=False, debug=False, num_devices=8)

    x_d = nc.declare_dram_parameter("x", [S, D], F32R, isOutput=False)
    w1_d = nc.declare_dram_parameter("w1g", [D, D], FP8, isOutput=False)
    b1_d = nc.declare_dram_parameter("b1p", [D], FP32, isOutput=False)
    w2_d = nc.declare_dram_parameter("w2", [D, D], FP8, isOutput=False)
    wg_d = nc.declare_dram_parameter("wg", [D2, D2], FP8, isOutput=False)
    tri_d = nc.declare_dram_parameter("tri", [P, P], F32R, isOutput=False)
    iden_d = nc.declare_dram_parameter("iden", [P, P], F32R, isOutput=False)
    sel_d = nc.declare_dram_parameter("sel", [P, P], F32R, isOutput=False)
    idenb_d = nc.declare_dram_parameter("idenb", [P, P], BF16, isOutput=False)
    inv_d = nc.declare_dram_parameter("invpos", [P, NBLK], FP32, isOutput=False)
    if has_b2:
        b2_d = nc.declare_dram_parameter("b2", [D], FP32, isOutput=False)
    if has_bg:
        bg_d = nc.declare_dram_parameter("bg", [D2], FP32, isOutput=False)

    gated_d = nc.declare_dram_parameter("gated", [S, D], FP32, isOutput=True)
    aout_d = nc.declare_dram_parameter("avg_out", [S, D], FP32, isOutput=True)

    x_r = x_d[:].rearrange("(n p) d -> p n d", p=P)        # [128, 16, 1024]
    aout_r = aout_d[:].rearrange("(n p) d -> p n d", p=P)
    gated_r = gated_d[:].rearrange("(n p) d -> p n d", p=P)
    w1_r = w1_d[:].rearrange("(c p) f -> p c f", p=P)      # [128, 8, 1024]
    w2_r = w2_d[:].rearrange("(c p) f -> p c f", p=P)
    wg_r = wg_d[:].rearrange("(c p) j -> p c j", p=P)      # [128, 16, 2048]

    with tile.TileContext(nc) as tc, ExitStack() as ctx:
        const = ctx.enter_context(tc.tile_pool(name="const", bufs=1))

        mm_ps = ctx.enter_context(tc.tile_pool(name="mm_ps", bufs=5, space="PSUM"))
        tr_ps = ctx.enter_context(tc.tile_pool(name="tr_ps", bufs=3, space="PSUM"))

        w12 = ctx.enter_context(tc.tile_pool(name="w12", bufs=1))
        xq_p = ctx.enter_context(tc.tile_pool(name="xq", bufs=6))
        avgq_p = ctx.enter_context(tc.tile_pool(name="avgq", bufs=3))
        zq_p = ctx.enter_context(tc.tile_pool(name="zq", bufs=2))
        xT_p = ctx.enter_context(tc.tile_pool(name="xT", bufs=4))
        aoT_p = ctx.enter_context(tc.tile_pool(name="aoT", bufs=3))
        lnT_p = ctx.enter_context(tc.tile_pool(name="lnT", bufs=3))
        intT_p = ctx.enter_context(tc.tile_pool(name="intT", bufs=2))
        aoq_p = ctx.enter_context(tc.tile_pool(name="aoq", bufs=3))
        stat_p = ctx.enter_context(tc.tile_pool(name="stat", bufs=6))
        incl_p = ctx.enter_context(tc.tile_pool(name="incl", bufs=3))
        sig_p = ctx.enter_context(tc.tile_pool(name="sig", bufs=4))
        g_p = ctx.enter_context(tc.tile_pool(name="g", bufs=2))

        def transpose_blk(src_ap, src_dt, dst_tile, dst_scol, on_dve=False):
            """Transpose a [128, 1024] fp32-width block into
            dst_tile[:, :, dst_scol:+128] as fp8.

            8 PE transposes batched 4-per-PSUM-bank, evicted (and cast to the
            dst dtype) on ScalarE. DMA-rounded f32r sources stream at 1.5
            cyc/row; compute-produced fp32 sources at 2 cyc/row."""
            iden = {F32R: iden_sb, FP32: idenf_sb, BF16: idenb_sb}[src_dt]
            for h in range(2):
                ptr = tr_ps.tile([P, 512], src_dt, tag="tr")
                for j in range(4):
                    k = 4 * h + j
                    nc.tensor.transpose(
                        ptr[:, j * P : (j + 1) * P],
                        src_ap[:, k * P : (k + 1) * P],
                        iden,
                    )
                ev = ptr[:].rearrange("p (j s) -> p j s", j=4)
                if src_dt is F32R:
                    ev = ev.bitcast(FP32)
                out_ap = dst_tile[:, 4 * h : 4 * h + 4, dst_scol : dst_scol + P]
                if on_dve:
                    nc.vector.tensor_copy(out=out_ap, in_=ev)
                else:
                    nc.scalar.copy(out=out_ap, in_=ev)

        x_tiles = {}

        def issue_x(qq):
            if qq >= NCHUNK:
                return
            t = xq_p.tile([P, CB, D], F32R)
            for bb in range(CB):
                nc.sync.dma_start(out=t[:, bb, :], in_=x_r[:, qq * CB + bb, :])
            x_tiles[qq] = t

        iden_sb = const.tile([P, P], F32R)
        nc.sync.dma_start(out=iden_sb, in_=iden_d[:])
        idenf_sb = const.tile([P, P], FP32)
        nc.sync.dma_start(out=idenf_sb, in_=iden_d[:].bitcast(FP32))
        idenb_sb = const.tile([P, P], BF16)
        nc.sync.dma_start(out=idenb_sb, in_=idenb_d[:])
        inv_sb = const.tile([P, NBLK], FP32)
        nc.sync.dma_start(out=inv_sb, in_=inv_d[:])
        b1t_sb = const.tile([P, KC], FP32)
        nc.sync.dma_start(out=b1t_sb, in_=b1_d[:].rearrange("(c p) -> p c", p=P))
        # int32 seed constant for the DVE fast-inverse-sqrt (keeps Sqrt off
        # ScalarE so the whole kernel fits one ACT table set — no mid-kernel
        # LoadActFuncSet switch before the gating sigmoids)
        magic_sb = const.tile([P, 1], mybir.dt.int32)
        nc.vector.memset(magic_sb, 0x5F3759DF)
        if has_b2:
            b2r_sb = const.tile([P, D], FP32)
            nc.sync.dma_start(out=b2r_sb, in_=b2_d[None, :].to_broadcast([P, D]))
        if has_bg:
            bgr_sb = const.tile([P, D2], FP32)
            nc.sync.dma_start(out=bgr_sb, in_=bg_d[None, :].to_broadcast([P, D2]))

        # fp32r operands may be DMA'd directly when the buffer dtype is f32r
        tri_rsb = const.tile([P, P], F32R)
        nc.sync.dma_start(out=tri_rsb, in_=tri_d[:])
        tri_r = tri_rsb[:]
        # sel rows 96..127: only row 127 is ones — a K=32 matmul against the
        # evicted PSUM window [96:128] broadcasts the inclusive prefix row
        sel_sb = const.tile([P, P], F32R)
        nc.sync.dma_start(out=sel_sb, in_=sel_d[:])

        issue_x(0)
        issue_x(1)

        w1_sb = w12.tile([P, KC, D], FP8)
        nc.sync.dma_start(out=w1_sb, in_=w1_r)
        w2_sb = w12.tile([P, KC, D], FP8)
        nc.sync.dma_start(out=w2_sb, in_=w2_r)
        issue_x(2)
        # full gating weight resident (gating is interleaved per chunk)
        wg_sb = w12.tile([P, 2 * KC, D2], FP8)
        for kh in range(4):
            nc.sync.dma_start(
                out=wg_sb[:, kh * 4 : (kh + 1) * 4, :],
                in_=wg_r[:, kh * 4 : (kh + 1) * 4, :],
            )

        prev_incl = [None]
        state = {}

        def stage_A(q, b):
            """Cumsum + serial carry + scale-evict for block b of chunk q.

            Issued so the cross-engine carry hops (PE matmuls -> DVE window
            evict -> PE sel-bcast) sit near the head of every engine queue."""
            if b == 0:
                x_q = x_tiles.pop(q)
                issue_x(q + 3)
                avg_t = avgq_p.tile([P, CB, D], FP32)
                xT_t = xT_p.tile([P, KC, CS], FP8)
                state[("avg", q)] = avg_t
                state[("xT", q)] = xT_t
                state[("xq", q)] = x_q
            x_q = state[("xq", q)]
            avg_q = state[("avg", q)]

            i = q * CB + b
            # in-block cumsum + carry bcast, scaled to cumulative average.
            # Row 127 of the fully-accumulated PSUM IS the inclusive
            # prefix; the aligned window [64:128] is evicted on DVE and
            # the next block's bcast selects row 127 via a K=64 matmul
            # whose stationary sel matrix has ones only in that row.
            cur_incl = None
            if i < NBLK - 1:
                cur_incl = incl_p.tile([P, D], F32R, tag="incl")
            for c in range(2):
                cs = slice(c * 512, (c + 1) * 512)
                ps = mm_ps.tile([P, 512], FP32, tag="mm")
                nc.tensor.matmul(
                    ps, lhsT=tri_r, rhs=x_q[:, b, cs],
                    start=True, stop=(i == 0),
                )
                if i > 0:
                    nc.tensor.matmul(
                        ps, lhsT=sel_sb[64:P, 0:P],
                        rhs=prev_incl[0][64:P, cs],
                        start=False, stop=True,
                    )
                if i < NBLK - 1:
                    nc.vector.tensor_copy(
                        out=cur_incl[64:P, cs], in_=ps[64:P, :]
                    )
                nc.scalar.mul(out=avg_q[:, b, cs], in_=ps, mul=inv_sb[:, i : i + 1])
            if i < NBLK - 1:
                prev_incl[0] = cur_incl

        def stage_A2(q):
            """x transposes for chunk q (bulk PE work, issued late)."""
            x_q = state[("xq", q)]
            xT_q = state[("xT", q)]
            for b in range(CB):
                transpose_blk(x_q[:, b, :], F32R, xT_q, b * P, on_dve=True)

        def stage_B(q):
            """LayerNorm + normalized-activation transpose for chunk q."""
            avg_q = state[("avg", q)]
            z_q = zq_p.tile([P, CB, D], BF16)
            for b in range(CB):
                st = stat_p.tile([P, 2, 6], FP32, tag="st")
                for g in range(2):
                    nc.vector.bn_stats(
                        out=st[:, g, :], in_=avg_q[:, b, g * 512 : (g + 1) * 512]
                    )
                mv = stat_p.tile([P, 2], FP32, tag="mv")
                nc.vector.bn_aggr(out=mv, in_=st)
                # rstd = 1/sqrt(var+eps) on DVE only: bit-hack seed + Newton
                y = stat_p.tile([P, 1], FP32, tag="y")
                nc.vector.tensor_scalar(
                    out=y, in0=mv[:, 1:2], scalar1=EPS, scalar2=None, op0=ALU.add
                )
                r0b = stat_p.tile([P, 1], mybir.dt.int32, tag="r0b")
                nc.vector.tensor_scalar(
                    out=r0b, in0=y[:].bitcast(mybir.dt.int32), scalar1=1,
                    scalar2=None, op0=ALU.logical_shift_right,
                )
                nc.vector.tensor_tensor(
                    out=r0b, in0=magic_sb, in1=r0b, op=ALU.subtract
                )
                rstd = r0b[:].bitcast(FP32)
                t = stat_p.tile([P, 1], FP32, tag="t")
                for _ in range(3):
                    nc.vector.tensor_tensor(out=t, in0=rstd, in1=rstd, op=ALU.mult)
                    nc.vector.tensor_tensor(out=t, in0=t, in1=y, op=ALU.mult)
                    nc.vector.tensor_scalar(
                        out=t, in0=t, scalar1=-0.5, scalar2=1.5,
                        op0=ALU.mult, op1=ALU.add,
                    )
                    nc.vector.tensor_tensor(out=rstd, in0=rstd, in1=t, op=ALU.mult)
                nc.vector.tensor_scalar(
                    out=z_q[:, b, :], in0=avg_q[:, b, :],
                    scalar1=mv[:, 0:1], scalar2=rstd,
                    op0=ALU.subtract, op1=ALU.mult,
                )
                if has_b2:
                    nc.gpsimd.tensor_add(
                        out=avg_q[:, b, :], in0=avg_q[:, b, :], in1=b2r_sb
                    )

            lnT_q = lnT_p.tile([P, KC, CS], FP8)
            for b in range(CB):
                transpose_blk(z_q[:, b, :], BF16, lnT_q, b * P, on_dve=True)
            state[("lnT", q)] = lnT_q

        def stage_C(q):
            """FFN1 + FFN2 + residual + avg_out store/transpose for chunk q."""
            avg_q = state.pop(("avg", q))
            lnT_q = state.pop(("lnT", q))
            # FFN1: interT[f, s] = relu(w1'.T-chunks @ lnT + b1')/WS
            # (weights are host-scaled by WS; interT is stored /WS so FFN2's
            #  WS-scaled w2 cancels it -- PSUM2 comes out unscaled.)
            intT_q = intT_p.tile([P, KC, CS], FP8)
            for fc in range(KC):
                ps = mm_ps.tile([P, 512], FP32, tag="mm")
                for k in range(0, KC, 2):
                    nc.tensor.matmul(
                        ps[:, :CS],
                        lhsT=w1_sb[:, k : k + 2, fc * P : (fc + 1) * P],
                        rhs=lnT_q[:, k : k + 2, :],
                        start=(k == 0), stop=(k == KC - 2),
                        perf_mode=DR,
                    )
                nc.scalar.activation(
                    out=intT_q[:, fc, :], in_=ps[:, :CS],
                    func=AF.Relu, bias=b1t_sb[:, fc : fc + 1],
                    scale=1.0 / (WS * WS),
                )

            # FFN2 + residual: avg_out = interT.T @ w2 + (avg + b2)
            ao_q = aoq_p.tile([P, CB, D], F32R)
            aoT_q = aoT_p.tile([P, KC, CS], FP8)
            for b in range(CB):
                i = q * CB + b
                for dc in range(2):
                    ds_ = slice(dc * 512, (dc + 1) * 512)
                    ps = mm_ps.tile([P, 512], FP32, tag="mm")
                    for f in range(0, KC, 2):
                        nc.tensor.matmul(
                            ps,
                            lhsT=intT_q[:, f : f + 2, b * P : (b + 1) * P],
                            rhs=w2_sb[:, f : f + 2, ds_],
                            start=(f == 0), stop=(f == KC - 2),
                            perf_mode=DR,
                        )
                    nc.vector.tensor_add(
                        out=ao_q[:, b, ds_], in0=ps, in1=avg_q[:, b, ds_]
                    )
                nc.sync.dma_start(out=aout_r[:, i, :], in_=ao_q[:, b, :].bitcast(FP32))
                transpose_blk(ao_q[:, b, :], F32R, aoT_q, b * P)
            state[("ao", q)] = ao_q
            state[("aoT", q)] = aoT_q

        def stage_D(q):
            """Gating matmuls + sigmoid + elementwise combine for chunk q."""
            xT_q = state.pop(("xT", q))
            ao_q = state.pop(("ao", q))
            aoT_q = state.pop(("aoT", q))
            x_q = state.pop(("xq", q))
            for b in range(CB):
                i = q * CB + b
                scol = slice(b * P, (b + 1) * P)
                for dh in range(2):  # output feature half (512 wide)
                    ds_ = slice(dh * 512, (dh + 1) * 512)
                    ps_pair = []
                    for wi in range(2):
                        ps = mm_ps.tile([P, 512], FP32, tag="mm")
                        jcol = slice(wi * D + dh * 512, wi * D + dh * 512 + 512)
                        for k in range(0, 2 * KC, 2):
                            lhs = (
                                xT_q[:, k : k + 2, scol]
                                if k < KC
                                else aoT_q[:, k - KC : k - KC + 2, scol]
                            )
                            nc.tensor.matmul(
                                ps, lhsT=lhs, rhs=wg_sb[:, k : k + 2, jcol],
                                start=(k == 0), stop=(k == 2 * KC - 2),
                                perf_mode=DR,
                            )
                        ps_pair.append(ps)
                    sig_in = sig_p.tile([P, 512], FP32, tag="sig")
                    sig_fg = sig_p.tile([P, 512], FP32, tag="sig")
                    if has_bg:
                        nc.vector.tensor_add(
                            out=sig_in, in0=ps_pair[0], in1=bgr_sb[:, ds_]
                        )
                        nc.scalar.activation(
                            out=sig_in, in_=sig_in, func=AF.Sigmoid, scale=1.0 / WS
                        )
                        nc.vector.tensor_add(
                            out=sig_fg, in0=ps_pair[1],
                            in1=bgr_sb[:, D + dh * 512 : D + (dh + 1) * 512],
                        )
                        nc.scalar.activation(
                            out=sig_fg, in_=sig_fg, func=AF.Sigmoid, scale=1.0 / WS
                        )
                    else:
                        nc.scalar.activation(
                            out=sig_in, in_=ps_pair[0], func=AF.Sigmoid,
                            scale=1.0 / WS,
                        )
                        nc.scalar.activation(
                            out=sig_fg, in_=ps_pair[1], func=AF.Sigmoid,
                            scale=1.0 / WS,
                        )

                    m1 = g_p.tile([P, 512], FP32, tag="m1")
                    nc.gpsimd.tensor_mul(out=m1, in0=sig_in, in1=x_q[:, b, ds_])
                    m2 = g_p.tile([P, 512], FP32, tag="m2")
                    nc.gpsimd.tensor_mul(out=m2, in0=sig_fg, in1=ao_q[:, b, ds_])
                    nc.vector.tensor_add(out=m1, in0=m1, in1=m2)
                    nc.sync.dma_start(
                        out=gated_d[i * P : (i + 1) * P, ds_], in_=m1
                    )

        # software-pipelined issue order: stage X of chunk t is issued before
        # stage X+1 of chunk t-1, so latency-critical ops lead every queue
        for t in range(NCHUNK + 3):
            if t < NCHUNK:
                stage_A(t, 0)
            if 3 <= t:
                stage_D(t - 3)
            if t < NCHUNK:
                stage_A(t, 1)
            if 2 <= t < NCHUNK + 2:
                stage_C(t - 2)
            if 1 <= t < NCHUNK + 1:
                stage_B(t - 1)
            if t < NCHUNK:
                stage_A2(t)

    nc.compile()
    return nc


def host_inputs(x, w1, b1, w2, b2, ln_g, ln_b, wg, bg):
    """Fold LN affine params into w1/b1, precompute constants, cast weights."""
    x = np.asarray(x, np.float32)
    w1 = np.asarray(w1, np.float32)
    w2 = np.asarray(w2, np.float32)
    wg = np.asarray(wg, np.float32)
    ln_g = np.asarray(ln_g, np.float32)
    ln_b = np.asarray(ln_b, np.float32)
    b1 = np.asarray(b1, np.float32)

    w1g = (ln_g[:, None] * w1 * WS).astype(ml_dtypes.float8_e4m3)
    b1p = ((b1 + ln_b @ w1) / WS).astype(np.float32)
    tri = np.triu(np.ones((P, P), np.float32))
    iden = np.eye(P, dtype=np.float32)  # loaded as both f32r and fp32
    pos = np.arange(S, dtype=np.float64).reshape(NBLK, P).T  # [P, NBLK]
    invpos = (1.0 / (pos + 1.0)).astype(np.float32)

    base = {
        "x": None,  # per-core
        "ones": np.ones((P, P), np.float32),
        "w1g": w1g,
        "b1p": b1p,
        "w2": (w2 * WS).astype(ml_dtypes.float8_e4m3),
        "wg": (wg * WS).astype(ml_dtypes.float8_e4m3),
        "tri": tri,
        "iden": iden,
        "sel": (np.arange(P)[:, None] == P - 1).astype(np.float32) * np.ones((1, P), np.float32),
        "idenb": np.eye(P, dtype=ml_dtypes.bfloat16),
        "invpos": invpos,
    }
    has_b2 = bool(np.any(b2))
    has_bg = bool(np.any(bg))
    if has_b2:
        base["b2"] = np.asarray(b2, np.float32)
    if has_bg:
        base["bg"] = (np.asarray(bg, np.float32) * WS).astype(np.float32)
    return base, has_b2, has_bg


_prog_cache = {}


def kernel(x, w1, b1, w2, b2, ln_g, ln_b, wg, bg):
    x = np.asarray(x, np.float32)
    assert x.shape == (B, S, D), x.shape
    base, has_b2, has_bg = host_inputs(x, w1, b1, w2, b2, ln_g, ln_b, wg, bg)

    key = (has_b2, has_bg)
    if key not in _prog_cache:
        _prog_cache[key] = build_program(has_b2, has_bg)
    nc = _prog_cache[key]

    in_maps = []
    for core in range(B):
        m = dict(base)
        m["x"] = np.ascontiguousarray(x[core])
        in_maps.append(m)

    res = run_bass_kernel_spmd(nc, in_maps, core_ids=list(range(B)))
    gated = np.stack([res.results[c]["gated"] for c in range(B)])
    avg_out = np.stack([res.results[c]["avg_out"] for c in range(B)])
    return gated, avg_out
